# revision 9
# baseline (speedup 1.0000x reference)
"""Trainium2 Bass kernel for nn_Attention_local (dense transformer block).

Data-parallel over batch: 8 images -> 8 NeuronCores, no collectives.

Layout notes:
- d-order per head: d = c_local*16 + (fy*4+fx); n-order: n = h1*32+w1.
  This is a consistent permutation of the reference head layout
  (attention is equivariant to it; phase 3 inverts it).
- Phase-major padded conv layout: 16 blocks (one per phase) of
  34x33-padded 32x32 coarse grids, so all 9 depthwise taps are plain
  strided fused-MACs with zero-padding handled by the pad cells.
"""

import numpy as np

HEADS = 4
C = 192
CO = 576
HW = 128
NPIX = HW * HW
G = 32
NP = G * G              # 1024
PH = 16
DH = 48 * PH            # 768
BLK = 34 * 33           # 1122
PADF = PH * BLK + 1     # 17953
EPS = 1e-12

_COMPILED = {}


def _build():
    import concourse.bass as bass
    import concourse.bacc as bacc
    import concourse.mybir as mybir
    from concourse.tile import TileContext
    from concourse.masks import make_identity
    from contextlib import ExitStack

    F32 = mybir.dt.float32
    F32R = mybir.dt.float32r
    AF = mybir.ActivationFunctionType
    ALU = mybir.AluOpType

    nc = bacc.Bacc("TRN2", target_bir_lowering=False, debug=False)

    x_d = nc.dram_tensor("x", [C, NPIX], F32, kind="ExternalInput")
    wT_d = nc.dram_tensor("wT", [C, CO], F32, kind="ExternalInput")
    qb_d = nc.dram_tensor("qb", [CO, 1], F32, kind="ExternalInput")
    dw9_d = nc.dram_tensor("dw9", [CO, 9], F32, kind="ExternalInput")
    db_d = nc.dram_tensor("db", [CO, 1], F32, kind="ExternalInput")
    pT_d = nc.dram_tensor("pT", [C, C], F32, kind="ExternalInput")
    pb_d = nc.dram_tensor("pb", [C, 1], F32, kind="ExternalInput")
    tpc_d = nc.dram_tensor("tpc", [C, 1], F32, kind="ExternalInput")
    ones_d = nc.dram_tensor("ones", [128, 8], F32, kind="ExternalInput")
    y_d = nc.dram_tensor("y", [C, NPIX], F32, kind="ExternalOutput")

    with TileContext(nc) as tc:
        with ExitStack() as es_all:
            dram = es_all.enter_context(tc.tile_pool(name="dram", bufs=1, space="DRAM"))
            qt_d = dram.tile([NP, C * PH], F32, tag="qt_d")
            kt_d = dram.tile([NP, C * PH], F32, tag="kt_d")
            v_d = dram.tile([HEADS * DH, NP], F32, tag="v_d")
            o_d = dram.tile([HEADS * DH, NP], F32, tag="o_d")
            kn_d = dram.tile([C, PH], F32, tag="kn_d")

            # ---------------- phase 1 ----------------
            with ExitStack() as es1:
                const = es1.enter_context(tc.tile_pool(name="const", bufs=1))
                wq_a = const.tile([128, CO], F32, tag="wq_a")
                wq_b = const.tile([64, CO], F32, tag="wq_b")
                nc.sync.dma_start(wq_a[:].bitcast(F32R), wT_d.ap()[0:128, :].bitcast(F32R))
                nc.sync.dma_start(wq_b[:].bitcast(F32R), wT_d.ap()[128:192, :].bitcast(F32R))
                ident = const.tile([128, 128], F32, tag="ident")
                make_identity(nc, ident)
                tpc = const.tile([128, 2], F32, tag="tpc")
                nc.sync.dma_start(tpc[0:128, 0:1], tpc_d.ap()[0:128, :])
                nc.sync.dma_start(tpc[0:64, 1:2], tpc_d.ap()[128:192, :])

                big = es1.enter_context(tc.tile_pool(name="big", bufs=1))
                cv = big.tile([128, PADF], F32, tag="cv")
                dw = big.tile([128, NPIX], F32, tag="dw")
                cvb = cv[:].rearrange("c (fy fx h w) -> c fy fx h w",
                                      fy=4, fx=4, h=34, w=33) if False else None
                # zero pad cells of cv once
                v16 = cv[:, 0:PH * BLK].rearrange("c (b h w) -> c b h w", b=PH, h=34)
                nc.gpsimd.memset(v16[:, :, 0:1, :], 0.0)      # top pad row
                nc.gpsimd.memset(v16[:, :, 33:34, :], 0.0)    # bottom pad row
                nc.gpsimd.memset(v16[:, :, :, 0:1], 0.0)      # left pad col
                nc.gpsimd.memset(cv[:, PH * BLK:PADF], 0.0)   # guard cell

                xpool = es1.enter_context(tc.tile_pool(name="xp", bufs=2))
                cpsum = es1.enter_context(tc.tile_pool(name="cps", bufs=4, space="PSUM"))
                tpsum = es1.enter_context(tc.tile_pool(name="tps", bufs=2, space="PSUM"))
                wpool = es1.enter_context(tc.tile_pool(name="wp", bufs=2))
                npool = es1.enter_context(tc.tile_pool(name="np", bufs=2))
                spool = es1.enter_context(tc.tile_pool(name="sp", bufs=2))
                gpool = es1.enter_context(tc.tile_pool(name="gp", bufs=1))

                CHUNKS = [
                    (128, [(0, 0, 128, 0)]),
                    (128, [(0, 0, 64, 128), (1, 64, 64, 0)]),
                    (128, [(1, 0, 128, 64)]),
                    (128, [(2, 0, 128, 0)]),
                    (64, [(2, 0, 64, 128)]),
                ]

                def fx_groups(dx):
                    if dx == 0:
                        return [(0, 4, 0, 0)]
                    if dx == 1:
                        return [(0, 3, 1, 0), (3, 1, -3, 1)]
                    return [(0, 1, 3, -1), (1, 3, -1, 0)]

                for m, (cnt, pieces) in enumerate(CHUNKS):
                    w16 = wpool.tile([128, 16], F32, tag="w16")
                    nc.sync.dma_start(w16[0:cnt, 0:9], dw9_d.ap()[m * 128:m * 128 + cnt, :])
                    nc.sync.dma_start(w16[0:cnt, 9:10], db_d.ap()[m * 128:m * 128 + cnt, :])
                    nc.sync.dma_start(w16[0:cnt, 10:11], qb_d.ap()[m * 128:m * 128 + cnt, :])
                    dwt = w16[:, 0:9]
                    dbt = w16[:, 9:10]
                    qbt = w16[:, 10:11]

                    # ---- 1x1 conv ----
                    cvv = cv[0:cnt, 0:PH * BLK].rearrange(
                        "c (fy fx h w) -> c fy w fx h", fy=4, fx=4, h=34, w=33)
                    for xi in range(16):
                        xa = xpool.tile([128, 1024], F32, tag="xa")
                        xb = xpool.tile([64, 1024], F32, tag="xb")
                        nc.sync.dma_start(
                            xa[:].bitcast(F32R),
                            x_d.ap()[0:128, xi * 1024:(xi + 1) * 1024].bitcast(F32R))
                        nc.sync.dma_start(
                            xb[:].bitcast(F32R),
                            x_d.ap()[128:192, xi * 1024:(xi + 1) * 1024].bitcast(F32R))
                        for nsub in range(2):
                            h1 = xi * 2 + nsub
                            ps = cpsum.tile([128, 512], F32, tag="cps")
                            nc.tensor.matmul(
                                ps[0:cnt, :],
                                wq_a[:, m * 128:m * 128 + cnt].bitcast(F32R),
                                xa[:, nsub * 512:(nsub + 1) * 512].bitcast(F32R),
                                start=True, stop=False)
                            nc.tensor.matmul(
                                ps[0:cnt, :],
                                wq_b[:, m * 128:m * 128 + cnt].bitcast(F32R),
                                xb[:, nsub * 512:(nsub + 1) * 512].bitcast(F32R),
                                start=False, stop=True)
                            dst = cvv[:, :, 1:33, :, h1 + 1:h1 + 2]
                            nc.scalar.activation(dst, ps[0:cnt, :], AF.Identity,
                                                 bias=w16[0:cnt, 10:11])

                    # ---- depthwise conv ----
                    cvi = cv[0:cnt, 0:PH * BLK].rearrange(
                        "c (fy fx h w) -> c fy fx h w", fy=4, fx=4, h=34, w=33)
                    dwv = dw[0:cnt, :].rearrange(
                        "c (fy fx h w) -> c fy fx h w", fy=4, fx=4, h=32, w=32)
                    nc.scalar.activation(dw[0:cnt, :], cvi[:, :, :, 1:33, 1:33],
                                         AF.Identity, bias=w16[0:cnt, 9:10],
                                         scale=w16[0:cnt, 4:5])
                    for ti, (dy, dx) in enumerate(
                            (a, b) for a in (-1, 0, 1) for b in (-1, 0, 1)):
                        if (dy, dx) == (0, 0):
                            continue
                        for fy in range(4):
                            eng = nc.vector if fy < 3 else nc.gpsimd
                            fyp = fy + dy
                            cy = 1 if fyp > 3 else (-1 if fyp < 0 else 0)
                            fyp -= 4 * cy
                            for fx in range(4):
                                fxp = fx + dx
                                cx = 1 if fxp > 3 else (-1 if fxp < 0 else 0)
                                fxp -= 4 * cx
                                out = bass.AP(dw.tensor, (fy * 4 + fx) * NP,
                                              [[NPIX, cnt], [1, NP]])
                                soff = ((fyp * 4 + fxp) * BLK
                                        + (1 + cy) * 33 + 1 + cx)
                                src = bass.AP(cv.tensor, soff,
                                              [[PADF, cnt], [33, 32], [1, 32]])
                                if fy < 3:
                                    eng.scalar_tensor_tensor(
                                        out, src, w16[0:cnt, ti:ti + 1], out,
                                        op0=ALU.mult, op1=ALU.add)
                                else:
                                    gtmp = gpool.tile([128, NP], F32, tag="gtmp")
                                    nc.gpsimd.tensor_scalar_mul(
                                        gtmp[0:cnt, :], src, w16[0:cnt, ti:ti + 1])
                                    nc.gpsimd.tensor_add(out, out, gtmp[0:cnt, :])

                    # ---- norms / q scale / kninv ----
                    for (tens, r0, pc, c0) in pieces:
                        if tens == 2:
                            continue
                        n2 = npool.tile([128, PH], F32, tag="n2")
                        sc = gpool.tile([128, NP], F32, tag="sc")
                        for p in range(PH):
                            nc.scalar.activation(
                                sc[r0:r0 + pc, :],
                                dw[r0:r0 + pc, p * NP:(p + 1) * NP],
                                AF.Square, accum_out=n2[r0:r0 + pc, p:p + 1])
                        nc.scalar.sqrt(n2[r0:r0 + pc, :], n2[r0:r0 + pc, :])
                        nc.vector.tensor_scalar_max(
                            n2[r0:r0 + pc, :], n2[r0:r0 + pc, :], EPS)
                        nc.vector.reciprocal(n2[r0:r0 + pc, :], n2[r0:r0 + pc, :])
                        if tens == 0:
                            tslc = tpc[r0:r0 + pc, 0:1] if c0 < 128 else tpc[0:pc, 1:2]
                            nc.vector.tensor_scalar_mul(
                                n2[r0:r0 + pc, :], n2[r0:r0 + pc, :], tslc)
                            for p in range(PH):
                                nc.scalar.mul(
                                    dw[r0:r0 + pc, p * NP:(p + 1) * NP],
                                    dw[r0:r0 + pc, p * NP:(p + 1) * NP],
                                    n2[r0:r0 + pc, p:p + 1])
                        else:
                            nc.sync.dma_start(kn_d[c0:c0 + pc, :], n2[r0:r0 + pc, :])

                    # ---- q/k transposes -> staging -> DRAM ----
                    for (tens, r0, pc, c0) in pieces:
                        if tens == 2:
                            continue
                        tgt = qt_d if tens == 0 else kt_d
                        for nck in range(8):
                            stg = spool.tile([128, 2048], F32, tag="stg")
                            stgv = stg[:].rearrange("n (c q) -> n q c", q=PH)
                            for quad in range(4):
                                tp = tpsum.tile([128, 512], F32, tag="tps")
                                for qq in range(4):
                                    p = quad * 4 + qq
                                    nc.tensor.transpose(
                                        tp[:, qq * 128:qq * 128 + pc],
                                        dw[r0:r0 + pc,
                                           p * NP + nck * 128:p * NP + (nck + 1) * 128],
                                        ident[r0:r0 + pc, r0:r0 + pc])
                                eng = nc.scalar if quad % 2 == 0 else nc.vector
                                srcv = tp[:].rearrange("n (q c) -> n q c", q=4)[:, :, 0:pc]
                                dstv = stgv[:, quad * 4:(quad + 1) * 4, 0:pc]
                                if quad % 2 == 0:
                                    nc.scalar.copy(dstv, srcv)
                                else:
                                    nc.vector.tensor_copy(dstv, srcv)
                            nc.sync.dma_start(
                                tgt[nck * 128:(nck + 1) * 128, c0 * 16:(c0 + pc) * 16],
                                stg[:, 0:pc * 16])

                    # ---- v spill ----
                    for (tens, r0, pc, c0) in pieces:
                        if tens != 2:
                            continue
                        c = c0
                        r = r0
                        while c < c0 + pc:
                            h = c // 48
                            take = min(48 - (c % 48), c0 + pc - c)
                            a0 = h * DH + (c % 48) * 16
                            nc.sync.dma_start(v_d[a0:a0 + take * 16, :], dw[r:r + take, :])
                            c += take
                            r += take

            # ---------------- phase 2: attention ----------------
            with ExitStack() as es2:
                qpool = es2.enter_context(tc.tile_pool(name="qp", bufs=10))
                kpool = es2.enter_context(tc.tile_pool(name="kp", bufs=10))
                vpool = es2.enter_context(tc.tile_pool(name="vp", bufs=8))
                epool = es2.enter_context(tc.tile_pool(name="ep", bufs=7))
                apsum = es2.enter_context(tc.tile_pool(name="aps", bufs=1, space="PSUM"))
                opsum = es2.enter_context(tc.tile_pool(name="ops", bufs=2, space="PSUM"))
                knpool = es2.enter_context(tc.tile_pool(name="knp", bufs=2))
                odiv = es2.enter_context(tc.tile_pool(name="od", bufs=2))
                onep = es2.enter_context(tc.tile_pool(name="onep", bufs=1))
                ones1 = onep.tile([128, 8], F32, tag="ones1")
                nc.sync.dma_start(ones1[:].bitcast(F32R), ones_d.ap().bitcast(F32R))
                knf = kn_d[:].rearrange("a b -> (a b)")

                for h in range(HEADS):
                    qts, kts, vts, ets = [], [], [], []
                    for nck in range(8):
                        qt = qpool.tile([128, DH], F32, tag="qt")
                        nc.sync.dma_start(
                            qt[:].bitcast(F32R),
                            qt_d[nck * 128:(nck + 1) * 128,
                                 h * DH:(h + 1) * DH].bitcast(F32R))
                        qts.append(qt)
                        kt = kpool.tile([128, DH], F32, tag="kt")
                        nc.sync.dma_start(
                            kt[:].bitcast(F32R),
                            kt_d[nck * 128:(nck + 1) * 128,
                                 h * DH:(h + 1) * DH].bitcast(F32R))
                        kts.append(kt)
                    for ec in range(6):
                        vt = vpool.tile([128, NP], F32, tag="vt")
                        nc.sync.dma_start(
                            vt[:].bitcast(F32R),
                            v_d[h * DH + ec * 128:h * DH + (ec + 1) * 128, :].bitcast(F32R))
                        vts.append(vt)
                    for ec in range(6):
                        kn = knpool.tile([128, 1], F32, tag="kn")
                        s0 = (48 * h + ec * 8) * PH
                        nc.sync.dma_start(kn[:], knf[s0:s0 + 128])
                        pa = apsum.tile([128, DH], F32, tag="aps")
                        for nck in range(8):
                            st, sp = nck == 0, nck == 7
                            lhs = kts[nck][:, ec * 128:(ec + 1) * 128].bitcast(F32R)
                            nc.tensor.matmul(pa[:, 0:512], lhs,
                                             qts[nck][:, 0:512].bitcast(F32R),
                                             start=st, stop=sp)
                            nc.tensor.matmul(pa[:, 512:DH], lhs,
                                             qts[nck][:, 512:DH].bitcast(F32R),
                                             start=st, stop=sp)
                        et = epool.tile([128, DH], F32, tag="et")
                        nc.scalar.activation(et[:].bitcast(F32R), pa[:], AF.Exp,
                                             scale=kn[:])
                        ets.append(et)
                    for dc in range(6):
                        po = opsum.tile([128, 1536], F32, tag="ops")
                        for ec in range(6):
                            st, sp = ec == 0, ec == 5
                            lhs = ets[ec][:, dc * 128:(dc + 1) * 128].bitcast(F32R)
                            nc.tensor.matmul(po[:, 0:512], lhs,
                                             vts[ec][:, 0:512].bitcast(F32R),
                                             start=st, stop=sp)
                            nc.tensor.matmul(po[:, 512:1024], lhs,
                                             vts[ec][:, 512:1024].bitcast(F32R),
                                             start=st, stop=sp)
                            nc.tensor.matmul(po[:, 1024:1032], lhs,
                                             ones1[:].bitcast(F32R),
                                             start=st, stop=sp)
                        zr = odiv.tile([128, 1], F32, tag="zr")
                        nc.vector.tensor_scalar_add(zr[:], po[:, 1024:1025], 1.0)
                        nc.vector.reciprocal(zr[:], zr[:])
                        ot = odiv.tile([128, NP], F32, tag="ot")
                        nc.scalar.mul(ot[:], po[:, 0:1024], zr[:])
                        nc.sync.dma_start(
                            o_d[h * DH + dc * 128:h * DH + (dc + 1) * 128, :], ot[:])

            # ---------------- phase 3: projection ----------------
            with ExitStack() as es3:
                ppool = es3.enter_context(tc.tile_pool(name="pp", bufs=1))
                olpool = es3.enter_context(tc.tile_pool(name="olp", bufs=3))
                ppsum = es3.enter_context(tc.tile_pool(name="pps", bufs=4, space="PSUM"))
                pwa = ppool.tile([128, C], F32, tag="pwa")
                pwb = ppool.tile([64, C], F32, tag="pwb")
                nc.sync.dma_start(pwa[:].bitcast(F32R), pT_d.ap()[0:128, :].bitcast(F32R))
                nc.sync.dma_start(pwb[:].bitcast(F32R), pT_d.ap()[128:192, :].bitcast(F32R))
                pba = ppool.tile([128, 1], F32, tag="pba")
                pbb = ppool.tile([64, 1], F32, tag="pbb")
                nc.sync.dma_start(pba[:], pb_d.ap()[0:128, :])
                nc.sync.dma_start(pbb[:], pb_d.ap()[128:192, :])
                ya = ppool.tile([128, NPIX], F32, tag="ya")
                yb = ppool.tile([64, NPIX], F32, tag="yb")
                ov = o_d[:].rearrange("(a p) n -> a p n", p=PH)

                for p in range(PH):
                    fy, fx = p // 4, p % 4
                    oa = olpool.tile([128, NP], F32, tag="oa")
                    ob = olpool.tile([64, NP], F32, tag="ob")
                    nc.sync.dma_start(oa[:].bitcast(F32R),
                                      ov[0:128, p:p + 1, :].bitcast(F32R))
                    nc.sync.dma_start(ob[:].bitcast(F32R),
                                      ov[128:192, p:p + 1, :].bitcast(F32R))
                    for (yt, pb, m0, mc) in ((ya, pba, 0, 128), (yb, pbb, 128, 64)):
                        ytv = yt[0:mc, :].rearrange(
                            "c (h g w f) -> c h w g f", h=32, g=4, w=32, f=4)
                        for nh in range(2):
                            ps = ppsum.tile([128, 512], F32, tag="pps")
                            nc.tensor.matmul(ps[0:mc, :],
                                             pwa[:, m0:m0 + mc].bitcast(F32R),
                                             oa[:, nh * 512:(nh + 1) * 512].bitcast(F32R),
                                             start=True, stop=False)
                            nc.tensor.matmul(ps[0:mc, :],
                                             pwb[:, m0:m0 + mc].bitcast(F32R),
                                             ob[:, nh * 512:(nh + 1) * 512].bitcast(F32R),
                                             start=False, stop=True)
                            dst = ytv[:, nh * 16:(nh + 1) * 16, :, fy:fy + 1, fx:fx + 1]
                            nc.scalar.activation(dst, ps[0:mc, :], AF.Identity,
                                                 bias=pb[0:mc, :])
                nc.sync.dma_start(y_d.ap()[0:128, :], ya[:])
                nc.sync.dma_start(y_d.ap()[128:192, :], yb[:])

    nc.compile()
    return nc


def kernel(**inputs):
    import concourse.bass_utils as bu

    x = np.asarray(inputs["x"], np.float32)
    qkv_w = np.asarray(inputs["qkv_w"], np.float32)
    qkv_b = np.asarray(inputs["qkv_b"], np.float32)
    dw_w = np.asarray(inputs["dw_w"], np.float32)
    dw_b = np.asarray(inputs["dw_b"], np.float32)
    proj_w = np.asarray(inputs["proj_w"], np.float32)
    proj_b = np.asarray(inputs["proj_b"], np.float32)
    temp = np.asarray(inputs["temperature"], np.float32).reshape(HEADS)

    if "nc" not in _COMPILED:
        _COMPILED["nc"] = _build()
    nc = _COMPILED["nc"]

    common = {
        "wT": np.ascontiguousarray(qkv_w.T),
        "qb": np.ascontiguousarray(qkv_b.reshape(CO, 1)),
        "dw9": np.ascontiguousarray(dw_w.reshape(CO, 9)),
        "db": np.ascontiguousarray(dw_b.reshape(CO, 1)),
        "pT": np.ascontiguousarray(proj_w.T),
        "pb": np.ascontiguousarray(proj_b.reshape(C, 1)),
        "tpc": np.ascontiguousarray(np.repeat(temp, 48).reshape(C, 1)),
        "ones": np.ones((128, 8), np.float32),
    }
    in_maps = [
        {"x": np.ascontiguousarray(x[b].reshape(C, NPIX)), **common}
        for b in range(x.shape[0])
    ]
    res = bu.run_bass_kernel_spmd(nc, in_maps, core_ids=list(range(len(in_maps))))
    out = np.stack([r["y"].reshape(C, HW, HW) for r in res.results])
    return out.astype(np.float32)


# revision 12
# speedup vs baseline: 1.0703x; 1.0703x over previous
"""Trainium2 Bass kernel for nn_Attention_local (dense transformer block).

Data-parallel over batch: 8 images -> 8 NeuronCores, no collectives.

Layout notes:
- d-order per head: d = c_local*16 + (fy*4+fx); n-order: n = h1*32+w1.
  This is a consistent permutation of the reference head layout
  (attention is equivariant to it; phase 3 inverts it).
- Phase-major padded conv layout: 16 blocks (one per phase) of
  34x33-padded 32x32 coarse grids, so all 9 depthwise taps are plain
  strided fused-MACs with zero-padding handled by the pad cells.
"""

import numpy as np

HEADS = 4
C = 192
CO = 576
HW = 128
NPIX = HW * HW
G = 32
NP = G * G              # 1024
PH = 16
DH = 48 * PH            # 768
BLK = 34 * 33           # 1122 (unsplit, unused)
BLK2 = 18 * 33          # 594: half-image padded block (16 h1 + 2 halo rows)
PADF2 = PH * BLK2 + 1   # 9505
HNP = 512               # n per half per phase
EPS = 1e-12

_COMPILED = {}


def _build():
    import concourse.bass as bass
    import concourse.bacc as bacc
    import concourse.mybir as mybir
    from concourse.tile import TileContext
    from concourse.masks import make_identity
    from contextlib import ExitStack

    F32 = mybir.dt.float32
    F32R = mybir.dt.float32r
    AF = mybir.ActivationFunctionType
    ALU = mybir.AluOpType

    nc = bacc.Bacc("TRN2", target_bir_lowering=False, debug=False)

    x_d = nc.dram_tensor("x", [C, NPIX], F32, kind="ExternalInput")
    wT_d = nc.dram_tensor("wT", [C, CO], F32, kind="ExternalInput")
    qb_d = nc.dram_tensor("qb", [CO, 1], F32, kind="ExternalInput")
    dw9_d = nc.dram_tensor("dw9", [CO, 9], F32, kind="ExternalInput")
    db_d = nc.dram_tensor("db", [CO, 1], F32, kind="ExternalInput")
    pT_d = nc.dram_tensor("pT", [C, C], F32, kind="ExternalInput")
    pb_d = nc.dram_tensor("pb", [C, 1], F32, kind="ExternalInput")
    tpc_d = nc.dram_tensor("tpc", [C, 1], F32, kind="ExternalInput")
    ones_d = nc.dram_tensor("ones", [128, 8], F32, kind="ExternalInput")
    y_d = nc.dram_tensor("y", [C, NPIX], F32, kind="ExternalOutput")

    with TileContext(nc) as tc:
        with ExitStack() as es_all:
            dram = es_all.enter_context(tc.tile_pool(name="dram", bufs=1, space="DRAM"))
            qt_d = dram.tile([NP, C * PH], F32, tag="qt_d")
            kt_d = dram.tile([NP, C * PH], F32, tag="kt_d")
            v_d = dram.tile([HEADS * DH, NP], F32, tag="v_d")
            o_d = dram.tile([HEADS * DH, NP], F32, tag="o_d")
            kn_d = dram.tile([C, PH], F32, tag="kn_d")

            # ---------------- phase 1 ----------------
            with ExitStack() as es1:
                const = es1.enter_context(tc.tile_pool(name="const", bufs=1))
                wq_a = const.tile([128, CO], F32, tag="wq_a")
                wq_b = const.tile([64, CO], F32, tag="wq_b")
                nc.sync.dma_start(wq_a[:].bitcast(F32R), wT_d.ap()[0:128, :].bitcast(F32R))
                nc.sync.dma_start(wq_b[:].bitcast(F32R), wT_d.ap()[128:192, :].bitcast(F32R))
                ident = const.tile([128, 128], F32, tag="ident")
                make_identity(nc, ident)
                tpc = const.tile([128, 2], F32, tag="tpc")
                nc.sync.dma_start(tpc[0:128, 0:1], tpc_d.ap()[0:128, :])
                nc.sync.dma_start(tpc[0:64, 1:2], tpc_d.ap()[128:192, :])

                big = es1.enter_context(tc.tile_pool(name="big", bufs=1))
                cva = big.tile([128, PADF2], F32, tag="cva")
                cvb = big.tile([128, PADF2], F32, tag="cvb")
                dwa1 = big.tile([128, 12 * HNP], F32, tag="dwa1")
                dwa2 = big.tile([128, 4 * HNP], F32, tag="dwa2")
                dwb1 = big.tile([128, 12 * HNP], F32, tag="dwb1")
                dwb2 = big.tile([128, 4 * HNP], F32, tag="dwb2")
                CVH = [cva, cvb]
                DWH = [[dwa1, dwa2], [dwb1, dwb2]]

                def dw_ap(hf, p, cnt, r0=0, n0=0, nn=HNP):
                    t = DWH[hf][0] if p < 12 else DWH[hf][1]
                    off = (p if p < 12 else p - 12) * HNP + n0
                    fs = 12 * HNP if p < 12 else 4 * HNP
                    return bass.AP(t.tensor, r0 * fs + off, [[fs, cnt], [1, nn]])

                # zero pad cells: left cols everywhere; top zero row of A; bottom of B
                for cvh in (cva, cvb):
                    vh = cvh[:, 0:PH * BLK2].rearrange(
                        "c (b h w) -> c b h w", b=PH, h=18)
                    nc.gpsimd.memset(vh[:, :, :, 0:1], 0.0)
                    nc.gpsimd.memset(cvh[:, PH * BLK2:PADF2], 0.0)
                va0 = cva[:, 0:PH * BLK2].rearrange("c (b h w) -> c b h w", b=PH, h=18)
                vb0 = cvb[:, 0:PH * BLK2].rearrange("c (b h w) -> c b h w", b=PH, h=18)
                nc.gpsimd.memset(va0[:, :, 0:1, :], 0.0)
                nc.gpsimd.memset(vb0[:, :, 17:18, :], 0.0)

                xpool = es1.enter_context(tc.tile_pool(name="xp", bufs=2))
                cpsum = es1.enter_context(tc.tile_pool(name="cps", bufs=4, space="PSUM"))
                tpsum = es1.enter_context(tc.tile_pool(name="tps", bufs=2, space="PSUM"))
                wpool = es1.enter_context(tc.tile_pool(name="wp", bufs=2))
                npool = es1.enter_context(tc.tile_pool(name="np", bufs=2))
                spool = es1.enter_context(tc.tile_pool(name="sp", bufs=2))
                gpool = es1.enter_context(tc.tile_pool(name="gp", bufs=1))

                CHUNKS = [
                    (128, [(0, 0, 128, 0)]),
                    (128, [(0, 0, 64, 128), (1, 64, 64, 0)]),
                    (128, [(1, 0, 128, 64)]),
                    (128, [(2, 0, 128, 0)]),
                    (64, [(2, 0, 64, 128)]),
                ]

                def fx_groups(dx):
                    if dx == 0:
                        return [(0, 4, 0, 0)]
                    if dx == 1:
                        return [(0, 3, 1, 0), (3, 1, -3, 1)]
                    return [(0, 1, 3, -1), (1, 3, -1, 0)]

                for m, (cnt, pieces) in enumerate(CHUNKS):
                    w16 = wpool.tile([128, 16], F32, tag="w16")
                    nc.sync.dma_start(w16[0:cnt, 0:9], dw9_d.ap()[m * 128:m * 128 + cnt, :])
                    nc.sync.dma_start(w16[0:cnt, 9:10], db_d.ap()[m * 128:m * 128 + cnt, :])
                    nc.sync.dma_start(w16[0:cnt, 10:11], qb_d.ap()[m * 128:m * 128 + cnt, :])
                    dwt = w16[:, 0:9]
                    dbt = w16[:, 9:10]
                    qbt = w16[:, 10:11]

                    # ---- 1x1 conv ----
                    cvva = cva[0:cnt, 0:PH * BLK2].rearrange(
                        "c (fy fx h w) -> c fy w fx h", fy=4, fx=4, h=18, w=33)
                    cvvb = cvb[0:cnt, 0:PH * BLK2].rearrange(
                        "c (fy fx h w) -> c fy w fx h", fy=4, fx=4, h=18, w=33)
                    for xi in range(16):
                        xa = xpool.tile([128, 1024], F32, tag="xa")
                        xb = xpool.tile([64, 1024], F32, tag="xb")
                        nc.sync.dma_start(
                            xa[:].bitcast(F32R),
                            x_d.ap()[0:128, xi * 1024:(xi + 1) * 1024].bitcast(F32R))
                        nc.sync.dma_start(
                            xb[:].bitcast(F32R),
                            x_d.ap()[128:192, xi * 1024:(xi + 1) * 1024].bitcast(F32R))
                        for nsub in range(2):
                            h1 = xi * 2 + nsub
                            ps = cpsum.tile([128, 512], F32, tag="cps")
                            nc.tensor.matmul(
                                ps[0:cnt, :],
                                wq_a[:, m * 128:m * 128 + cnt].bitcast(F32R),
                                xa[:, nsub * 512:(nsub + 1) * 512].bitcast(F32R),
                                start=True, stop=False)
                            nc.tensor.matmul(
                                ps[0:cnt, :],
                                wq_b[:, m * 128:m * 128 + cnt].bitcast(F32R),
                                xb[:, nsub * 512:(nsub + 1) * 512].bitcast(F32R),
                                start=False, stop=True)
                            dsts = []
                            if h1 <= 16:
                                dsts.append(cvva[:, :, 1:33, :, h1 + 1:h1 + 2])
                            if h1 >= 15:
                                dsts.append(cvvb[:, :, 1:33, :, h1 - 15:h1 - 14])
                            for dst in dsts:
                                nc.scalar.activation(dst, ps[0:cnt, :], AF.Identity,
                                                     bias=w16[0:cnt, 10:11])

                    # ---- depthwise conv ----
                    for hf in range(2):
                        cvh = CVH[hf]
                        cviv = cvh[0:cnt, 0:PH * BLK2].rearrange(
                            "c (fy fx h w) -> c fy fx h w", fy=4, fx=4, h=18, w=33)
                        nc.scalar.activation(
                            DWH[hf][0][0:cnt, :], cviv[:, 0:3, :, 1:17, 1:33],
                            AF.Identity, bias=w16[0:cnt, 9:10],
                            scale=w16[0:cnt, 4:5])
                        nc.scalar.activation(
                            DWH[hf][1][0:cnt, :], cviv[:, 3:4, :, 1:17, 1:33],
                            AF.Identity, bias=w16[0:cnt, 9:10],
                            scale=w16[0:cnt, 4:5])
                        for ti, (dy, dx) in enumerate(
                                (a, b) for a in (-1, 0, 1) for b in (-1, 0, 1)):
                            if (dy, dx) == (0, 0):
                                continue
                            for fy in range(4):
                                eng = nc.vector if fy < 3 else nc.gpsimd
                                fyp = fy + dy
                                cy = 1 if fyp > 3 else (-1 if fyp < 0 else 0)
                                fyp -= 4 * cy
                                for fx in range(4):
                                    fxp = fx + dx
                                    cx = 1 if fxp > 3 else (-1 if fxp < 0 else 0)
                                    fxp -= 4 * cx
                                    out = dw_ap(hf, fy * 4 + fx, cnt)
                                    soff = ((fyp * 4 + fxp) * BLK2
                                            + (1 + cy) * 33 + 1 + cx)
                                    srch = bass.AP(cvh.tensor, soff,
                                                   [[PADF2, cnt], [33, 16], [1, 32]])
                                    if fy < 3:
                                        eng.scalar_tensor_tensor(
                                            out, srch, w16[0:cnt, ti:ti + 1], out,
                                            op0=ALU.mult, op1=ALU.add)
                                    else:
                                        gtmp = gpool.tile([128, HNP], F32, tag="gtmp")
                                        nc.gpsimd.tensor_scalar_mul(
                                            gtmp[0:cnt, :], srch,
                                            w16[0:cnt, ti:ti + 1])
                                        nc.gpsimd.tensor_add(out, out,
                                                             gtmp[0:cnt, :])

                    # ---- norms / q scale / kninv ----
                    for (tens, r0, pc, c0) in pieces:
                        if tens == 2:
                            continue
                        n2 = npool.tile([128, PH], F32, tag="n2")
                        n2b = npool.tile([128, PH], F32, tag="n2b")
                        sc = gpool.tile([128, HNP], F32, tag="sc")
                        for p in range(PH):
                            nc.scalar.activation(
                                sc[r0:r0 + pc, :], dw_ap(0, p, pc, r0),
                                AF.Square, accum_out=n2[r0:r0 + pc, p:p + 1])
                            nc.scalar.activation(
                                sc[r0:r0 + pc, :], dw_ap(1, p, pc, r0),
                                AF.Square, accum_out=n2b[r0:r0 + pc, p:p + 1])
                        nc.vector.tensor_add(n2[r0:r0 + pc, :], n2[r0:r0 + pc, :],
                                             n2b[r0:r0 + pc, :])
                        nc.scalar.sqrt(n2[r0:r0 + pc, :], n2[r0:r0 + pc, :])
                        nc.vector.tensor_scalar_max(
                            n2[r0:r0 + pc, :], n2[r0:r0 + pc, :], EPS)
                        nc.vector.reciprocal(n2[r0:r0 + pc, :], n2[r0:r0 + pc, :])
                        if tens == 0:
                            tslc = tpc[r0:r0 + pc, 0:1] if c0 < 128 else tpc[0:pc, 1:2]
                            nc.vector.tensor_scalar_mul(
                                n2[r0:r0 + pc, :], n2[r0:r0 + pc, :], tslc)
                            for p in range(PH):
                                for hf in range(2):
                                    nc.scalar.mul(
                                        dw_ap(hf, p, pc, r0), dw_ap(hf, p, pc, r0),
                                        n2[r0:r0 + pc, p:p + 1])
                        else:
                            nc.sync.dma_start(kn_d[c0:c0 + pc, :], n2[r0:r0 + pc, :])

                    # ---- q/k transposes -> staging -> DRAM ----
                    for (tens, r0, pc, c0) in pieces:
                        if tens == 2:
                            continue
                        tgt = qt_d if tens == 0 else kt_d
                        for nck in range(8):
                            stg = spool.tile([128, 2048], F32, tag="stg")
                            stgv = stg[:].rearrange("n (c q) -> n q c", q=PH)
                            for quad in range(4):
                                tp = tpsum.tile([128, 512], F32, tag="tps")
                                for qq in range(4):
                                    p = quad * 4 + qq
                                    hf = 0 if nck < 4 else 1
                                    n0 = (nck % 4) * 128
                                    tsrc = dw_ap(hf, p, pc, r0, n0, 128)
                                    nc.tensor.transpose(
                                        tp[:, qq * 128:qq * 128 + pc],
                                        tsrc, ident[r0:r0 + pc, r0:r0 + pc])
                                eng = nc.scalar if quad % 2 == 0 else nc.vector
                                srcv = tp[:].rearrange("n (q c) -> n q c", q=4)[:, :, 0:pc]
                                dstv = stgv[:, quad * 4:(quad + 1) * 4, 0:pc]
                                if quad % 2 == 0:
                                    nc.scalar.copy(dstv, srcv)
                                else:
                                    nc.vector.tensor_copy(dstv, srcv)
                            nc.sync.dma_start(
                                tgt[nck * 128:(nck + 1) * 128, c0 * 16:(c0 + pc) * 16],
                                stg[:, 0:pc * 16])

                    # ---- v spill ----
                    for (tens, r0, pc, c0) in pieces:
                        if tens != 2:
                            continue
                        c = c0
                        r = r0
                        vv = v_d[:].rearrange("(a p) n -> a p n", p=PH)
                        while c < c0 + pc:
                            h = c // 48
                            take = min(48 - (c % 48), c0 + pc - c)
                            a0 = h * 48 + (c % 48)
                            nc.sync.dma_start(vv[a0:a0 + take, 0:12, 0:HNP],
                                              dwa1[r:r + take, :])
                            nc.sync.dma_start(vv[a0:a0 + take, 12:16, 0:HNP],
                                              dwa2[r:r + take, :])
                            nc.sync.dma_start(vv[a0:a0 + take, 0:12, HNP:NP],
                                              dwb1[r:r + take, :])
                            nc.sync.dma_start(vv[a0:a0 + take, 12:16, HNP:NP],
                                              dwb2[r:r + take, :])
                            c += take
                            r += take

            # ---------------- phase 2: attention ----------------
            with ExitStack() as es2:
                qpool = es2.enter_context(tc.tile_pool(name="qp", bufs=10))
                kpool = es2.enter_context(tc.tile_pool(name="kp", bufs=10))
                vpool = es2.enter_context(tc.tile_pool(name="vp", bufs=8))
                epool = es2.enter_context(tc.tile_pool(name="ep", bufs=7))
                apsum = es2.enter_context(tc.tile_pool(name="aps", bufs=2, space="PSUM"))
                opsum = es2.enter_context(tc.tile_pool(name="ops", bufs=1, space="PSUM"))
                knpool = es2.enter_context(tc.tile_pool(name="knp", bufs=2))
                odiv = es2.enter_context(tc.tile_pool(name="od", bufs=2))
                onep = es2.enter_context(tc.tile_pool(name="onep", bufs=1))
                ones1 = onep.tile([128, 8], F32, tag="ones1")
                nc.sync.dma_start(ones1[:].bitcast(F32R), ones_d.ap().bitcast(F32R))
                knf = kn_d[:].rearrange("a b -> (a b)")

                for h in range(HEADS):
                    qts, kts, vts, ets = [], [], [], []
                    for nck in range(8):
                        qt = qpool.tile([128, DH], F32, tag="qt")
                        nc.sync.dma_start(
                            qt[:].bitcast(F32R),
                            qt_d[nck * 128:(nck + 1) * 128,
                                 h * DH:(h + 1) * DH].bitcast(F32R))
                        qts.append(qt)
                        kt = kpool.tile([128, DH], F32, tag="kt")
                        nc.sync.dma_start(
                            kt[:].bitcast(F32R),
                            kt_d[nck * 128:(nck + 1) * 128,
                                 h * DH:(h + 1) * DH].bitcast(F32R))
                        kts.append(kt)
                    for ec in range(6):
                        vt = vpool.tile([128, NP], F32, tag="vt")
                        nc.sync.dma_start(
                            vt[:].bitcast(F32R),
                            v_d[h * DH + ec * 128:h * DH + (ec + 1) * 128, :].bitcast(F32R))
                        vts.append(vt)
                    for ec in range(6):
                        kn = knpool.tile([128, 1], F32, tag="kn")
                        s0 = (48 * h + ec * 8) * PH
                        nc.sync.dma_start(kn[:], knf[s0:s0 + 128])
                        pa = apsum.tile([128, DH], F32, tag="aps")
                        for nck in range(8):
                            st, sp = nck == 0, nck == 7
                            lhs = kts[nck][:, ec * 128:(ec + 1) * 128].bitcast(F32R)
                            nc.tensor.matmul(pa[:, 0:512], lhs,
                                             qts[nck][:, 0:512].bitcast(F32R),
                                             start=st, stop=sp)
                            nc.tensor.matmul(pa[:, 512:DH], lhs,
                                             qts[nck][:, 512:DH].bitcast(F32R),
                                             start=st, stop=sp)
                        et = epool.tile([128, DH], F32, tag="et")
                        nc.scalar.activation(et[:].bitcast(F32R), pa[:], AF.Exp,
                                             scale=kn[:])
                        ets.append(et)
                    for dc in range(6):
                        po = opsum.tile([128, 1536], F32, tag="ops")
                        for ec in range(6):
                            st, sp = ec == 0, ec == 5
                            lhs = ets[ec][:, dc * 128:(dc + 1) * 128].bitcast(F32R)
                            nc.tensor.matmul(po[:, 0:512], lhs,
                                             vts[ec][:, 0:512].bitcast(F32R),
                                             start=st, stop=sp)
                            nc.tensor.matmul(po[:, 512:1024], lhs,
                                             vts[ec][:, 512:1024].bitcast(F32R),
                                             start=st, stop=sp)
                            nc.tensor.matmul(po[:, 1024:1032], lhs,
                                             ones1[:].bitcast(F32R),
                                             start=st, stop=sp)
                        zr = odiv.tile([128, 1], F32, tag="zr")
                        nc.vector.tensor_scalar_add(zr[:], po[:, 1024:1025], 1.0)
                        nc.vector.reciprocal(zr[:], zr[:])
                        ot = odiv.tile([128, NP], F32, tag="ot")
                        nc.scalar.mul(ot[:], po[:, 0:1024], zr[:])
                        nc.sync.dma_start(
                            o_d[h * DH + dc * 128:h * DH + (dc + 1) * 128, :], ot[:])

            # ---------------- phase 3: projection ----------------
            with ExitStack() as es3:
                ppool = es3.enter_context(tc.tile_pool(name="pp", bufs=1))
                olpool = es3.enter_context(tc.tile_pool(name="olp", bufs=3))
                ppsum = es3.enter_context(tc.tile_pool(name="pps", bufs=4, space="PSUM"))
                pwa = ppool.tile([128, C], F32, tag="pwa")
                pwb = ppool.tile([64, C], F32, tag="pwb")
                nc.sync.dma_start(pwa[:].bitcast(F32R), pT_d.ap()[0:128, :].bitcast(F32R))
                nc.sync.dma_start(pwb[:].bitcast(F32R), pT_d.ap()[128:192, :].bitcast(F32R))
                pba = ppool.tile([128, 1], F32, tag="pba")
                pbb = ppool.tile([64, 1], F32, tag="pbb")
                nc.sync.dma_start(pba[:], pb_d.ap()[0:128, :])
                nc.sync.dma_start(pbb[:], pb_d.ap()[128:192, :])
                ya = ppool.tile([128, NPIX], F32, tag="ya")
                yb = ppool.tile([64, NPIX], F32, tag="yb")
                ov = o_d[:].rearrange("(a p) n -> a p n", p=PH)

                for p in range(PH):
                    fy, fx = p // 4, p % 4
                    oa = olpool.tile([128, NP], F32, tag="oa")
                    ob = olpool.tile([64, NP], F32, tag="ob")
                    nc.sync.dma_start(oa[:].bitcast(F32R),
                                      ov[0:128, p:p + 1, :].bitcast(F32R))
                    nc.sync.dma_start(ob[:].bitcast(F32R),
                                      ov[128:192, p:p + 1, :].bitcast(F32R))
                    for (yt, pb, m0, mc) in ((ya, pba, 0, 128), (yb, pbb, 128, 64)):
                        ytv = yt[0:mc, :].rearrange(
                            "c (h g w f) -> c h w g f", h=32, g=4, w=32, f=4)
                        for nh in range(2):
                            ps = ppsum.tile([128, 512], F32, tag="pps")
                            nc.tensor.matmul(ps[0:mc, :],
                                             pwa[:, m0:m0 + mc].bitcast(F32R),
                                             oa[:, nh * 512:(nh + 1) * 512].bitcast(F32R),
                                             start=True, stop=False)
                            nc.tensor.matmul(ps[0:mc, :],
                                             pwb[:, m0:m0 + mc].bitcast(F32R),
                                             ob[:, nh * 512:(nh + 1) * 512].bitcast(F32R),
                                             start=False, stop=True)
                            dst = ytv[:, nh * 16:(nh + 1) * 16, :, fy:fy + 1, fx:fx + 1]
                            nc.scalar.activation(dst, ps[0:mc, :], AF.Identity,
                                                 bias=pb[0:mc, :])
                nc.sync.dma_start(y_d.ap()[0:128, :], ya[:])
                nc.sync.dma_start(y_d.ap()[128:192, :], yb[:])

    nc.compile()
    return nc


def kernel(**inputs):
    import concourse.bass_utils as bu

    x = np.asarray(inputs["x"], np.float32)
    qkv_w = np.asarray(inputs["qkv_w"], np.float32)
    qkv_b = np.asarray(inputs["qkv_b"], np.float32)
    dw_w = np.asarray(inputs["dw_w"], np.float32)
    dw_b = np.asarray(inputs["dw_b"], np.float32)
    proj_w = np.asarray(inputs["proj_w"], np.float32)
    proj_b = np.asarray(inputs["proj_b"], np.float32)
    temp = np.asarray(inputs["temperature"], np.float32).reshape(HEADS)

    if "nc" not in _COMPILED:
        _COMPILED["nc"] = _build()
    nc = _COMPILED["nc"]

    common = {
        "wT": np.ascontiguousarray(qkv_w.T),
        "qb": np.ascontiguousarray(qkv_b.reshape(CO, 1)),
        "dw9": np.ascontiguousarray(dw_w.reshape(CO, 9)),
        "db": np.ascontiguousarray(dw_b.reshape(CO, 1)),
        "pT": np.ascontiguousarray(proj_w.T),
        "pb": np.ascontiguousarray(proj_b.reshape(C, 1)),
        "tpc": np.ascontiguousarray(np.repeat(temp, 48).reshape(C, 1)),
        "ones": np.ones((128, 8), np.float32),
    }
    in_maps = [
        {"x": np.ascontiguousarray(x[b].reshape(C, NPIX)), **common}
        for b in range(x.shape[0])
    ]
    res = bu.run_bass_kernel_spmd(nc, in_maps, core_ids=list(range(len(in_maps))))
    out = np.stack([r["y"].reshape(C, HW, HW) for r in res.results])
    return out.astype(np.float32)


# revision 15
# speedup vs baseline: 5221.2334x; 4878.1715x over previous
"""Trainium2 Bass kernel for nn_Attention_local (dense transformer block).

Data-parallel over batch: 8 images -> 8 NeuronCores, no collectives.

Layout notes:
- d-order per head: d = c_local*16 + (fy*4+fx); n-order: n = h1*32+w1.
  This is a consistent permutation of the reference head layout
  (attention is equivariant to it; phase 3 inverts it).
- Phase-major padded conv layout: 16 blocks (one per phase) of
  34x33-padded 32x32 coarse grids, so all 9 depthwise taps are plain
  strided fused-MACs with zero-padding handled by the pad cells.
"""

import numpy as np

HEADS = 4
C = 192
CO = 576
HW = 128
NPIX = HW * HW
G = 32
NP = G * G              # 1024
PH = 16
DH = 48 * PH            # 768
BLK = 34 * 33           # 1122 (unsplit, unused)
BLK2 = 18 * 33          # 594: half-image padded block (16 h1 + 2 halo rows)
PADF2 = PH * BLK2 + 1   # 9505
HNP = 512               # n per half per phase
EPS = 1e-12

_COMPILED = {}


def _build():
    import concourse.bass as bass
    import concourse.bacc as bacc
    import concourse.mybir as mybir
    from concourse.tile import TileContext
    from concourse.masks import make_identity
    from contextlib import ExitStack

    F32 = mybir.dt.float32
    F32R = mybir.dt.float32r
    AF = mybir.ActivationFunctionType
    ALU = mybir.AluOpType

    nc = bacc.Bacc("TRN2", target_bir_lowering=False, debug=False)

    x_d = nc.dram_tensor("x", [C, NPIX], F32, kind="ExternalInput")
    wT_d = nc.dram_tensor("wT", [C, CO], F32, kind="ExternalInput")
    qb_d = nc.dram_tensor("qb", [CO, 1], F32, kind="ExternalInput")
    dw9_d = nc.dram_tensor("dw9", [CO, 9], F32, kind="ExternalInput")
    db_d = nc.dram_tensor("db", [CO, 1], F32, kind="ExternalInput")
    pT_d = nc.dram_tensor("pT", [C, C], F32, kind="ExternalInput")
    pb_d = nc.dram_tensor("pb", [C, 1], F32, kind="ExternalInput")
    tpc_d = nc.dram_tensor("tpc", [C, 1], F32, kind="ExternalInput")
    ones_d = nc.dram_tensor("ones", [128, 8], F32, kind="ExternalInput")
    y_d = nc.dram_tensor("y", [C, NPIX], F32, kind="ExternalOutput")

    with TileContext(nc) as tc:
        with ExitStack() as es_all:
            dram = es_all.enter_context(tc.tile_pool(name="dram", bufs=1, space="DRAM"))
            qt_d = dram.tile([NP, C * PH], F32, tag="qt_d")
            kt_d = dram.tile([NP, C * PH], F32, tag="kt_d")
            v_d = dram.tile([HEADS * DH, NP], F32, tag="v_d")
            o_d = dram.tile([HEADS * DH, NP], F32, tag="o_d")
            kn_d = dram.tile([C, PH], F32, tag="kn_d")

            # ---------------- phase 1 ----------------
            with ExitStack() as es1:
                const = es1.enter_context(tc.tile_pool(name="const", bufs=1))
                wq_a = const.tile([128, CO], F32, tag="wq_a")
                wq_b = const.tile([64, CO], F32, tag="wq_b")
                nc.sync.dma_start(wq_a[:].bitcast(F32R), wT_d.ap()[0:128, :].bitcast(F32R))
                nc.sync.dma_start(wq_b[:].bitcast(F32R), wT_d.ap()[128:192, :].bitcast(F32R))
                ident = const.tile([128, 128], F32, tag="ident")
                make_identity(nc, ident)
                tpc = const.tile([128, 2], F32, tag="tpc")
                nc.sync.dma_start(tpc[0:128, 0:1], tpc_d.ap()[0:128, :])
                nc.sync.dma_start(tpc[0:64, 1:2], tpc_d.ap()[128:192, :])

                big = es1.enter_context(tc.tile_pool(name="big", bufs=1))
                cva = big.tile([128, PADF2], F32, tag="cva")
                cvb = big.tile([128, PADF2], F32, tag="cvb")
                dwa1 = big.tile([128, 12 * HNP], F32, tag="dwa1")
                dwa2 = big.tile([128, 4 * HNP], F32, tag="dwa2")
                dwb1 = big.tile([128, 12 * HNP], F32, tag="dwb1")
                dwb2 = big.tile([128, 4 * HNP], F32, tag="dwb2")
                CVH = [cva, cvb]
                DWH = [[dwa1, dwa2], [dwb1, dwb2]]

                def dw_ap(hf, p, cnt, r0=0, n0=0, nn=HNP):
                    t = DWH[hf][0] if p < 12 else DWH[hf][1]
                    off = (p if p < 12 else p - 12) * HNP + n0
                    fs = 12 * HNP if p < 12 else 4 * HNP
                    return bass.AP(t.tensor, r0 * fs + off, [[fs, cnt], [1, nn]])

                # zero pad cells: left cols everywhere; top zero row of A; bottom of B
                for cvh in (cva, cvb):
                    vh = cvh[:, 0:PH * BLK2].rearrange(
                        "c (b h w) -> c b h w", b=PH, h=18)
                    nc.gpsimd.memset(vh[:, :, :, 0:1], 0.0)
                    nc.gpsimd.memset(cvh[:, PH * BLK2:PADF2], 0.0)
                va0 = cva[:, 0:PH * BLK2].rearrange("c (b h w) -> c b h w", b=PH, h=18)
                vb0 = cvb[:, 0:PH * BLK2].rearrange("c (b h w) -> c b h w", b=PH, h=18)
                nc.gpsimd.memset(va0[:, :, 0:1, :], 0.0)
                nc.gpsimd.memset(vb0[:, :, 17:18, :], 0.0)

                xpool = es1.enter_context(tc.tile_pool(name="xp", bufs=4))
                cpsum = es1.enter_context(tc.tile_pool(name="cps", bufs=5, space="PSUM"))
                tpsum = es1.enter_context(tc.tile_pool(name="tps", bufs=3, space="PSUM"))
                wpool = es1.enter_context(tc.tile_pool(name="wp", bufs=2))
                npool = es1.enter_context(tc.tile_pool(name="np", bufs=2))
                spool = es1.enter_context(tc.tile_pool(name="sp", bufs=3))
                gpool = es1.enter_context(tc.tile_pool(name="gp", bufs=1))

                CHUNKS = [
                    (128, [(0, 0, 128, 0)]),
                    (128, [(0, 0, 64, 128), (1, 64, 64, 0)]),
                    (128, [(1, 0, 128, 64)]),
                    (128, [(2, 0, 128, 0)]),
                    (64, [(2, 0, 64, 128)]),
                ]

                def fx_groups(dx):
                    if dx == 0:
                        return [(0, 4, 0, 0)]
                    if dx == 1:
                        return [(0, 3, 1, 0), (3, 1, -3, 1)]
                    return [(0, 1, 3, -1), (1, 3, -1, 0)]

                for m, (cnt, pieces) in enumerate(CHUNKS):
                    w16 = wpool.tile([128, 16], F32, tag="w16")
                    nc.sync.dma_start(w16[0:cnt, 0:9], dw9_d.ap()[m * 128:m * 128 + cnt, :])
                    nc.sync.dma_start(w16[0:cnt, 9:10], db_d.ap()[m * 128:m * 128 + cnt, :])
                    nc.sync.dma_start(w16[0:cnt, 10:11], qb_d.ap()[m * 128:m * 128 + cnt, :])
                    dwt = w16[:, 0:9]
                    dbt = w16[:, 9:10]
                    qbt = w16[:, 10:11]

                    # ---- 1x1 conv ----
                    cvva = cva[0:cnt, 0:PH * BLK2].rearrange(
                        "c (fy fx h w) -> c fy w fx h", fy=4, fx=4, h=18, w=33)
                    cvvb = cvb[0:cnt, 0:PH * BLK2].rearrange(
                        "c (fy fx h w) -> c fy w fx h", fy=4, fx=4, h=18, w=33)
                    for xi in range(16):
                        xa = xpool.tile([128, 1024], F32, tag="xa")
                        xb = xpool.tile([64, 1024], F32, tag="xb")
                        nc.sync.dma_start(
                            xa[:].bitcast(F32R),
                            x_d.ap()[0:128, xi * 1024:(xi + 1) * 1024].bitcast(F32R))
                        nc.sync.dma_start(
                            xb[:].bitcast(F32R),
                            x_d.ap()[128:192, xi * 1024:(xi + 1) * 1024].bitcast(F32R))
                        for nsub in range(2):
                            h1 = xi * 2 + nsub
                            ps = cpsum.tile([128, 512], F32, tag="cps")
                            nc.tensor.matmul(
                                ps[0:cnt, :],
                                wq_a[:, m * 128:m * 128 + cnt].bitcast(F32R),
                                xa[:, nsub * 512:(nsub + 1) * 512].bitcast(F32R),
                                start=True, stop=False)
                            nc.tensor.matmul(
                                ps[0:cnt, :],
                                wq_b[:, m * 128:m * 128 + cnt].bitcast(F32R),
                                xb[:, nsub * 512:(nsub + 1) * 512].bitcast(F32R),
                                start=False, stop=True)
                            dsts = []
                            if h1 <= 16:
                                dsts.append(cvva[:, :, 1:33, :, h1 + 1:h1 + 2])
                            if h1 >= 15:
                                dsts.append(cvvb[:, :, 1:33, :, h1 - 15:h1 - 14])
                            for dst in dsts:
                                nc.scalar.activation(dst, ps[0:cnt, :], AF.Identity,
                                                     bias=w16[0:cnt, 10:11])

                    # ---- depthwise conv ----
                    for hf in range(2):
                        cvh = CVH[hf]
                        cviv = cvh[0:cnt, 0:PH * BLK2].rearrange(
                            "c (fy fx h w) -> c fy fx h w", fy=4, fx=4, h=18, w=33)
                        nc.scalar.activation(
                            DWH[hf][0][0:cnt, :], cviv[:, 0:3, :, 1:17, 1:33],
                            AF.Identity, bias=w16[0:cnt, 9:10],
                            scale=w16[0:cnt, 4:5])
                        nc.scalar.activation(
                            DWH[hf][1][0:cnt, :], cviv[:, 3:4, :, 1:17, 1:33],
                            AF.Identity, bias=w16[0:cnt, 9:10],
                            scale=w16[0:cnt, 4:5])
                        for ti, (dy, dx) in enumerate(
                                (a, b) for a in (-1, 0, 1) for b in (-1, 0, 1)):
                            if (dy, dx) == (0, 0):
                                continue
                            for fy in range(4):
                                eng = nc.vector if fy < 3 else nc.gpsimd
                                fyp = fy + dy
                                cy = 1 if fyp > 3 else (-1 if fyp < 0 else 0)
                                fyp -= 4 * cy
                                for fx in range(4):
                                    fxp = fx + dx
                                    cx = 1 if fxp > 3 else (-1 if fxp < 0 else 0)
                                    fxp -= 4 * cx
                                    out = dw_ap(hf, fy * 4 + fx, cnt)
                                    soff = ((fyp * 4 + fxp) * BLK2
                                            + (1 + cy) * 33 + 1 + cx)
                                    srch = bass.AP(cvh.tensor, soff,
                                                   [[PADF2, cnt], [33, 16], [1, 32]])
                                    if fy < 3:
                                        eng.scalar_tensor_tensor(
                                            out, srch, w16[0:cnt, ti:ti + 1], out,
                                            op0=ALU.mult, op1=ALU.add)
                                    else:
                                        gtmp = gpool.tile([128, HNP], F32, tag="gtmp")
                                        nc.gpsimd.tensor_scalar_mul(
                                            gtmp[0:cnt, :], srch,
                                            w16[0:cnt, ti:ti + 1])
                                        nc.gpsimd.tensor_add(out, out,
                                                             gtmp[0:cnt, :])

                    # ---- norms / q scale / kninv ----
                    for (tens, r0, pc, c0) in pieces:
                        if tens == 2:
                            continue
                        n2 = npool.tile([128, PH], F32, tag="n2")
                        n2b = npool.tile([128, PH], F32, tag="n2b")
                        sc = gpool.tile([128, HNP], F32, tag="sc")
                        for p in range(PH):
                            nc.scalar.activation(
                                sc[r0:r0 + pc, :], dw_ap(0, p, pc, r0),
                                AF.Square, accum_out=n2[r0:r0 + pc, p:p + 1])
                            nc.scalar.activation(
                                sc[r0:r0 + pc, :], dw_ap(1, p, pc, r0),
                                AF.Square, accum_out=n2b[r0:r0 + pc, p:p + 1])
                        nc.vector.tensor_add(n2[r0:r0 + pc, :], n2[r0:r0 + pc, :],
                                             n2b[r0:r0 + pc, :])
                        nc.scalar.sqrt(n2[r0:r0 + pc, :], n2[r0:r0 + pc, :])
                        nc.vector.tensor_scalar_max(
                            n2[r0:r0 + pc, :], n2[r0:r0 + pc, :], EPS)
                        nc.vector.reciprocal(n2[r0:r0 + pc, :], n2[r0:r0 + pc, :])
                        if tens == 0:
                            tslc = tpc[r0:r0 + pc, 0:1] if c0 < 128 else tpc[0:pc, 1:2]
                            nc.vector.tensor_scalar_mul(
                                n2[r0:r0 + pc, :], n2[r0:r0 + pc, :], tslc)
                            for p in range(PH):
                                for hf in range(2):
                                    nc.scalar.mul(
                                        dw_ap(hf, p, pc, r0), dw_ap(hf, p, pc, r0),
                                        n2[r0:r0 + pc, p:p + 1])
                        else:
                            nc.sync.dma_start(kn_d[c0:c0 + pc, :], n2[r0:r0 + pc, :])

                    # ---- q/k transposes -> staging -> DRAM ----
                    for (tens, r0, pc, c0) in pieces:
                        if tens == 2:
                            continue
                        tgt = qt_d if tens == 0 else kt_d
                        for nck in range(8):
                            stg = spool.tile([128, 2048], F32, tag="stg")
                            stgv = stg[:].rearrange("n (c q) -> n q c", q=PH)
                            for quad in range(4):
                                tp = tpsum.tile([128, 512], F32, tag="tps")
                                for qq in range(4):
                                    p = quad * 4 + qq
                                    hf = 0 if nck < 4 else 1
                                    n0 = (nck % 4) * 128
                                    tsrc = dw_ap(hf, p, pc, r0, n0, 128)
                                    nc.tensor.transpose(
                                        tp[:, qq * 128:qq * 128 + pc],
                                        tsrc, ident[r0:r0 + pc, r0:r0 + pc])
                                eng = nc.scalar if quad % 2 == 0 else nc.vector
                                srcv = tp[:].rearrange("n (q c) -> n q c", q=4)[:, :, 0:pc]
                                dstv = stgv[:, quad * 4:(quad + 1) * 4, 0:pc]
                                if quad % 2 == 0:
                                    nc.scalar.copy(dstv, srcv)
                                else:
                                    nc.vector.tensor_copy(dstv, srcv)
                            nc.sync.dma_start(
                                tgt[nck * 128:(nck + 1) * 128, c0 * 16:(c0 + pc) * 16],
                                stg[:, 0:pc * 16])

                    # ---- v spill ----
                    for (tens, r0, pc, c0) in pieces:
                        if tens != 2:
                            continue
                        c = c0
                        r = r0
                        vv = v_d[:].rearrange("(a p) n -> a p n", p=PH)
                        while c < c0 + pc:
                            h = c // 48
                            take = min(48 - (c % 48), c0 + pc - c)
                            a0 = h * 48 + (c % 48)
                            nc.sync.dma_start(vv[a0:a0 + take, 0:12, 0:HNP],
                                              dwa1[r:r + take, :])
                            nc.sync.dma_start(vv[a0:a0 + take, 12:16, 0:HNP],
                                              dwa2[r:r + take, :])
                            nc.sync.dma_start(vv[a0:a0 + take, 0:12, HNP:NP],
                                              dwb1[r:r + take, :])
                            nc.sync.dma_start(vv[a0:a0 + take, 12:16, HNP:NP],
                                              dwb2[r:r + take, :])
                            c += take
                            r += take

            # ---------------- phase 2: attention ----------------
            with ExitStack() as es2:
                qpool = es2.enter_context(tc.tile_pool(name="qp", bufs=12))
                kpool = es2.enter_context(tc.tile_pool(name="kp", bufs=12))
                vpool = es2.enter_context(tc.tile_pool(name="vp", bufs=9))
                epool = es2.enter_context(tc.tile_pool(name="ep", bufs=7))
                apsum = es2.enter_context(tc.tile_pool(name="aps", bufs=2, space="PSUM"))
                opsum = es2.enter_context(tc.tile_pool(name="ops", bufs=1, space="PSUM"))
                knpool = es2.enter_context(tc.tile_pool(name="knp", bufs=2))
                odiv = es2.enter_context(tc.tile_pool(name="od", bufs=2))
                onep = es2.enter_context(tc.tile_pool(name="onep", bufs=1))
                ones1 = onep.tile([128, 8], F32, tag="ones1")
                nc.sync.dma_start(ones1[:].bitcast(F32R), ones_d.ap().bitcast(F32R))
                knf = kn_d[:].rearrange("a b -> (a b)")

                for h in range(HEADS):
                    qts, kts, vts, ets = [], [], [], []
                    for nck in range(8):
                        qt = qpool.tile([128, DH], F32, tag="qt")
                        nc.sync.dma_start(
                            qt[:].bitcast(F32R),
                            qt_d[nck * 128:(nck + 1) * 128,
                                 h * DH:(h + 1) * DH].bitcast(F32R))
                        qts.append(qt)
                        kt = kpool.tile([128, DH], F32, tag="kt")
                        nc.sync.dma_start(
                            kt[:].bitcast(F32R),
                            kt_d[nck * 128:(nck + 1) * 128,
                                 h * DH:(h + 1) * DH].bitcast(F32R))
                        kts.append(kt)
                    for ec in range(6):
                        vt = vpool.tile([128, NP], F32, tag="vt")
                        nc.sync.dma_start(
                            vt[:].bitcast(F32R),
                            v_d[h * DH + ec * 128:h * DH + (ec + 1) * 128, :].bitcast(F32R))
                        vts.append(vt)
                    for ec in range(6):
                        kn = knpool.tile([128, 1], F32, tag="kn")
                        s0 = (48 * h + ec * 8) * PH
                        nc.sync.dma_start(kn[:], knf[s0:s0 + 128])
                        pa = apsum.tile([128, DH], F32, tag="aps")
                        for nck in range(8):
                            st, sp = nck == 0, nck == 7
                            lhs = kts[nck][:, ec * 128:(ec + 1) * 128].bitcast(F32R)
                            nc.tensor.matmul(pa[:, 0:512], lhs,
                                             qts[nck][:, 0:512].bitcast(F32R),
                                             start=st, stop=sp)
                            nc.tensor.matmul(pa[:, 512:DH], lhs,
                                             qts[nck][:, 512:DH].bitcast(F32R),
                                             start=st, stop=sp)
                        et = epool.tile([128, DH], F32, tag="et")
                        nc.scalar.activation(et[:].bitcast(F32R), pa[:], AF.Exp,
                                             scale=kn[:])
                        ets.append(et)
                    for dc in range(6):
                        po = opsum.tile([128, 1536], F32, tag="ops")
                        for ec in range(6):
                            st, sp = ec == 0, ec == 5
                            lhs = ets[ec][:, dc * 128:(dc + 1) * 128].bitcast(F32R)
                            nc.tensor.matmul(po[:, 0:512], lhs,
                                             vts[ec][:, 0:512].bitcast(F32R),
                                             start=st, stop=sp)
                            nc.tensor.matmul(po[:, 512:1024], lhs,
                                             vts[ec][:, 512:1024].bitcast(F32R),
                                             start=st, stop=sp)
                            nc.tensor.matmul(po[:, 1024:1032], lhs,
                                             ones1[:].bitcast(F32R),
                                             start=st, stop=sp)
                        zr = odiv.tile([128, 1], F32, tag="zr")
                        nc.vector.tensor_scalar_add(zr[:], po[:, 1024:1025], 1.0)
                        nc.vector.reciprocal(zr[:], zr[:])
                        ot = odiv.tile([128, NP], F32, tag="ot")
                        nc.scalar.mul(ot[:], po[:, 0:1024], zr[:])
                        nc.sync.dma_start(
                            o_d[h * DH + dc * 128:h * DH + (dc + 1) * 128, :], ot[:])

            # ---------------- phase 3: projection ----------------
            with ExitStack() as es3:
                ppool = es3.enter_context(tc.tile_pool(name="pp", bufs=1))
                olpool = es3.enter_context(tc.tile_pool(name="olp", bufs=4))
                ppsum = es3.enter_context(tc.tile_pool(name="pps", bufs=4, space="PSUM"))
                pwa = ppool.tile([128, C], F32, tag="pwa")
                pwb = ppool.tile([64, C], F32, tag="pwb")
                nc.sync.dma_start(pwa[:].bitcast(F32R), pT_d.ap()[0:128, :].bitcast(F32R))
                nc.sync.dma_start(pwb[:].bitcast(F32R), pT_d.ap()[128:192, :].bitcast(F32R))
                pba = ppool.tile([128, 1], F32, tag="pba")
                pbb = ppool.tile([64, 1], F32, tag="pbb")
                nc.sync.dma_start(pba[:], pb_d.ap()[0:128, :])
                nc.sync.dma_start(pbb[:], pb_d.ap()[128:192, :])
                ya = ppool.tile([128, NPIX], F32, tag="ya")
                yb = ppool.tile([64, NPIX], F32, tag="yb")
                ov = o_d[:].rearrange("(a p) n -> a p n", p=PH)

                for p in range(PH):
                    fy, fx = p // 4, p % 4
                    oa = olpool.tile([128, NP], F32, tag="oa")
                    ob = olpool.tile([64, NP], F32, tag="ob")
                    nc.sync.dma_start(oa[:].bitcast(F32R),
                                      ov[0:128, p:p + 1, :].bitcast(F32R))
                    nc.sync.dma_start(ob[:].bitcast(F32R),
                                      ov[128:192, p:p + 1, :].bitcast(F32R))
                    for (yt, pb, m0, mc) in ((ya, pba, 0, 128), (yb, pbb, 128, 64)):
                        ytv = yt[0:mc, :].rearrange(
                            "c (h g w f) -> c h w g f", h=32, g=4, w=32, f=4)
                        for nh in range(2):
                            ps = ppsum.tile([128, 512], F32, tag="pps")
                            nc.tensor.matmul(ps[0:mc, :],
                                             pwa[:, m0:m0 + mc].bitcast(F32R),
                                             oa[:, nh * 512:(nh + 1) * 512].bitcast(F32R),
                                             start=True, stop=False)
                            nc.tensor.matmul(ps[0:mc, :],
                                             pwb[:, m0:m0 + mc].bitcast(F32R),
                                             ob[:, nh * 512:(nh + 1) * 512].bitcast(F32R),
                                             start=False, stop=True)
                            dst = ytv[:, nh * 16:(nh + 1) * 16, :, fy:fy + 1, fx:fx + 1]
                            nc.scalar.activation(dst, ps[0:mc, :], AF.Identity,
                                                 bias=pb[0:mc, :])
                nc.sync.dma_start(y_d.ap()[0:128, :], ya[:])
                nc.sync.dma_start(y_d.ap()[128:192, :], yb[:])

    nc.compile()
    return nc


def kernel(**inputs):
    import concourse.bass_utils as bu

    x = np.asarray(inputs["x"], np.float32)
    qkv_w = np.asarray(inputs["qkv_w"], np.float32)
    qkv_b = np.asarray(inputs["qkv_b"], np.float32)
    dw_w = np.asarray(inputs["dw_w"], np.float32)
    dw_b = np.asarray(inputs["dw_b"], np.float32)
    proj_w = np.asarray(inputs["proj_w"], np.float32)
    proj_b = np.asarray(inputs["proj_b"], np.float32)
    temp = np.asarray(inputs["temperature"], np.float32).reshape(HEADS)

    if "nc" not in _COMPILED:
        _COMPILED["nc"] = _build()
    nc = _COMPILED["nc"]

    common = {
        "wT": np.ascontiguousarray(qkv_w.T),
        "qb": np.ascontiguousarray(qkv_b.reshape(CO, 1)),
        "dw9": np.ascontiguousarray(dw_w.reshape(CO, 9)),
        "db": np.ascontiguousarray(dw_b.reshape(CO, 1)),
        "pT": np.ascontiguousarray(proj_w.T),
        "pb": np.ascontiguousarray(proj_b.reshape(C, 1)),
        "tpc": np.ascontiguousarray(np.repeat(temp, 48).reshape(C, 1)),
        "ones": np.ones((128, 8), np.float32),
    }
    in_maps = [
        {"x": np.ascontiguousarray(x[b].reshape(C, NPIX)), **common}
        for b in range(x.shape[0])
    ]
    res = bu.run_bass_kernel_spmd(nc, in_maps, core_ids=list(range(len(in_maps))))
    out = np.stack([r["y"].reshape(C, HW, HW) for r in res.results])
    return out.astype(np.float32)
